# revision 1
# baseline (speedup 1.0000x reference)
"""Bass/Trainium2 kernel for nn_GRUClassifier: 2-layer BiGRU + max-pool + MLP head.

Sharding: 8 cores = 4 batch groups x 2 L1-direction roles. Each core computes
BOTH L0 directions for its 16 sequences (duplicated across the role pair so no
cross-core exchange is needed), then one L1 direction, max-pool over time, and
the W1 partial of the classifier head. Host sums the two W1 partials per batch
group and applies relu + W2 (8.4 KFLOP per sample vs ~3.7 GFLOP on device).

All matmul operands fp16, accumulation fp32 in PSUM. Sequence reversal for the
backward direction is done by feeding the reversed token stream (host prep) so
every core runs the identical SPMD program; the only cross-order access is the
L1 "peer half" input projection, which is stored in produced order and read at
compile-time reversed offsets inside the unrolled recurrence.
"""
import os
import sys
import numpy as np

sys.path.insert(0, "/opt/trn_rl_repo")

B, T, E, H, V = 64, 256, 300, 512, 50000
EP = 384            # E padded to 3*128
G = 3 * H           # 1536 gate rows = 12 chunks of 128
BL = 16             # batch per core
NBLK = 8            # token blocks of 512 (= 32 steps * 16 batch)
SBLK = 16           # steps per xp/y block
NTOK = T * BL       # 4096

F16 = None          # set after imports in _build
F32 = None

_CACHE = {}


def _patch_drain():
    """walrus CoreV3 rejects CTRL (Drain) instructions with too many sem
    waits; split the tail-drain's waits across preceding sync nops."""
    from concourse import mybir
    from concourse.tile import TileContext
    from concourse.vector_clock import ScopedClock

    if getattr(TileContext, "_drain_patched", False):
        return
    MAXW = 1

    def _drain_and_barrier(self, tick_clock, wait_clock):
        drain_inst = self.nc.sync.drain()
        wait_clock.add_sem_waits(
            drain_inst.ins, ScopedClock({None: tick_clock.global_clock})
        )
        si = drain_inst.ins.sync_info
        if si is not None and si.on_wait and len(si.on_wait) > MAXW:
            waits = list(si.on_wait)
            si.on_wait = waits[:MAXW]
            for i in range(MAXW, len(waits), MAXW):
                nop = self.nc.sync.nop(nofuse=True, hint="drain_wait_split")
                nsi = nop.ins.sync_info
                if nsi is None:
                    nop.ins.sync_info = mybir.SyncInfo(
                        on_wait=waits[i : i + MAXW], on_update=[]
                    )
                else:
                    nsi.on_wait = waits[i : i + MAXW]
        self.nc.all_engine_barrier()
        assert self.sems is not None
        popped = self.nc._tile_sem_poison_stack.pop()
        assert popped is self._sem_poison
        self.nc.clear_and_free_semaphores(list(self.sems.allocated().values()))
        self.nc.all_engine_barrier()

    TileContext._drain_and_barrier = _drain_and_barrier
    TileContext._drain_patched = True


def _split_multiwaits(nc, mybir, maxw=1):
    """walrus CoreV2/V3 setupSyncWait rejects instructions with more than one
    sem wait; split extras onto preceding same-engine nops."""
    cnt = 0
    for fn in nc.m.functions:
        for bb in fn.blocks:
            insts = bb.instructions
            out = []
            changed = False
            for inst in insts:
                si = getattr(inst, "sync_info", None)
                eng = getattr(inst, "engine", None)
                if (
                    si is not None
                    and si.on_wait
                    and len(si.on_wait) > maxw
                    and eng is not None
                    and eng != mybir.EngineType.Unassigned
                ):
                    waits = list(si.on_wait)
                    for w in waits[:-maxw]:
                        nop = mybir.InstNoOp(
                            name=f"ws_nop_{cnt}", ins=[], outs=[]
                        )
                        cnt += 1
                        nop.engine = eng
                        nop.sync_info = mybir.SyncInfo(
                            on_wait=[w], on_update=[]
                        )
                        out.append(nop)
                    si.on_wait = waits[-maxw:]
                    changed = True
                out.append(inst)
            if changed:
                bb.instructions = out


def _build_nc():
    from concourse import bass, mybir
    from concourse.tile import TileContext

    _patch_drain()
    f16 = mybir.dt.float16
    f32 = mybir.dt.float32
    AF = mybir.ActivationFunctionType
    OP = mybir.AluOpType

    nc = bass.Bass(target_bir_lowering=False)

    def par(name, shape, dt=f16, out=False):
        return nc.declare_dram_parameter(name, list(shape), dt, isOutput=out)

    eT1 = par("eT1", [128, 3, NTOK])          # phase-1 embedded input (transposed)
    eT2 = par("eT2", [128, 3, NTOK])          # phase-2 (other direction's order)
    wih1 = par("wih1", [128, 3, G])           # L0 W_ih^T k-tiles, phase-1 dir
    wih2 = par("wih2", [128, 3, G])
    whh1 = par("whh1", [128, 4, G])           # L0 W_hh^T k-tiles
    whh2 = par("whh2", [128, 4, G])
    wa = par("wa", [128, 4, G])               # L1 W_ih^T, direct-source half
    wb = par("wb", [128, 4, G])               # L1 W_ih^T, reversed-source half
    whhL = par("whhL", [128, 4, G])
    bias1 = par("bias1", [128, 12], f32)      # xp bias per gate chunk (n: b_ih only)
    bias2 = par("bias2", [128, 12], f32)
    biasL = par("biasL", [128, 12], f32)
    nb1 = par("nb1", [128, 4], f32)           # b_hh n-gate chunks
    nb2 = par("nb2", [128, 4], f32)
    nbL = par("nbL", [128, 4], f32)
    w1h = par("w1h", [128, 4, 128])           # classifier W1 own-half^T k-tiles
    headout = par("headout", [128, 16], f32, out=True)

    xp1d = nc.dram_tensor("xp1d", [128, 12, NTOK], f16)
    xp2d = nc.dram_tensor("xp2d", [128, 12, NTOK], f16)
    xpad = nc.dram_tensor("xpad", [128, 12, NTOK], f16)
    xpbd = nc.dram_tensor("xpbd", [128, 12, NTOK], f16)
    y1d = nc.dram_tensor("y1d", [128, 4, NTOK], f16)
    y2d = nc.dram_tensor("y2d", [128, 4, NTOK], f16)

    with TileContext(nc) as tc:
        with (
            tc.tile_pool(name="wpool", bufs=1) as wp,
            tc.tile_pool(name="io", bufs=3) as io,
            tc.tile_pool(name="xpp", bufs=2) as xpp,
            tc.tile_pool(name="ew", bufs=2) as ew,
            tc.tile_pool(name="hp", bufs=2) as hp,
            tc.tile_pool(name="ps", bufs=2, space="PSUM") as ps,
            tc.tile_pool(name="psg", bufs=4, space="PSUM") as psg,
        ):
            # --- load all weights/biases into SBUF ---
            def load(p, shape, dt=f16):
                t = wp.tile(list(shape), dt, tag=p.name + "_sb")
                nc.sync.dma_start(out=t[:], in_=p[:])
                return t

            wih1_s = load(wih1, [128, 3, G])
            wih2_s = load(wih2, [128, 3, G])
            whh1_s = load(whh1, [128, 4, G])
            whh2_s = load(whh2, [128, 4, G])
            wa_s = load(wa, [128, 4, G])
            wb_s = load(wb, [128, 4, G])
            whhL_s = load(whhL, [128, 4, G])
            bias1_s = load(bias1, [128, 12], f32)
            bias2_s = load(bias2, [128, 12], f32)
            biasL_s = load(biasL, [128, 12], f32)
            nb1_s = load(nb1, [128, 4], f32)
            nb2_s = load(nb2, [128, 4], f32)
            nbL_s = load(nbL, [128, 4], f32)
            w1h_s = load(w1h, [128, 4, 128])

            def xp_gemm_blk(blk, src_dram, w_sb, kt, bias_sb, dst_dram):
                    sl = slice(blk * 512, (blk + 1) * 512)
                    et = io.tile([128, kt, 512], f16, tag="xg_in")
                    nc.sync.dma_start(out=et[:], in_=src_dram[:, :, sl])
                    for m in range(12):
                        p = ps.tile([128, 512], f32, tag="gemm_ps")
                        for k in range(kt):
                            nc.tensor.matmul(
                                p[:],
                                w_sb[:, k, m * 128 : (m + 1) * 128],
                                et[:, k, :],
                                start=(k == 0),
                                stop=(k == kt - 1),
                            )
                        xs = io.tile([128, 512], f16, tag="xg_out")
                        nc.scalar.activation(
                            xs[:], p[:], AF.Identity, bias=bias_sb[:, m : m + 1]
                        )
                        nc.sync.dma_start(out=dst_dram[:, m, sl], in_=xs[:])

            def xp_gemm(src_dram, w_sb, kt, bias_sb, dst_dram, tag):
                for blk in range(NBLK):
                    xp_gemm_blk(blk, src_dram, w_sb, kt, bias_sb, dst_dram)

            def recurrence(whh_sb, xp_dram, nb_sb, y_dram=None, xpb_dram=None,
                           pooled=None, side=None, tag="rc"):
                h = hp.tile([128, 4, 16], f16, tag="rc_h")
                nc.vector.memset(h[:], 0.0)
                yb = None
                for t in range(T):
                    if side and t in side:
                        for fn in side[t]:
                            fn()
                    blk, v = t // SBLK, t % SBLK
                    vs = slice(v * 16, (v + 1) * 16)
                    rv = SBLK - 1 - v
                    rvs = slice(rv * 16, (rv + 1) * 16)
                    if v == 0:
                        sl = slice(blk * 256, (blk + 1) * 256)
                        xpt = xpp.tile([128, 12, 256], f16, tag="rc_xpt")
                        nc.sync.dma_start(out=xpt[:], in_=xp_dram[:, :, sl])
                        if xpb_dram is not None:
                            rb = (T // SBLK) - 1 - blk
                            rsl = slice(rb * 256, (rb + 1) * 256)
                            xbt = xpp.tile([128, 12, 256], f16, tag="rc_xbt")
                            nc.sync.dma_start(out=xbt[:], in_=xpb_dram[:, :, rsl])
                        if y_dram is not None:
                            yb = io.tile([128, 4, 256], f16, tag="rc_yb")
                    pst = psg.tile([128, 12, 16], f32, tag="rc_ps")
                    for m in range(12):
                        out = pst[:, m, :]
                        for k in range(4):
                            nc.tensor.matmul(
                                out,
                                whh_sb[:, k, m * 128 : (m + 1) * 128],
                                h[:, k, :],
                                start=(k == 0),
                                stop=(k == 3),
                            )
                    def gsum(lo, hi, otag):
                        o = ew.tile([128, 4, 16], f32, tag=otag)
                        nc.vector.scalar_tensor_tensor(
                            out=o[:], in0=pst[:, lo:hi, :], scalar=1.0,
                            in1=xpt[:, lo:hi, vs], op0=OP.mult, op1=OP.add,
                        )
                        if xpb_dram is not None:
                            nc.vector.scalar_tensor_tensor(
                                out=o[:], in0=o[:], scalar=1.0,
                                in1=xbt[:, lo:hi, rvs], op0=OP.mult, op1=OP.add,
                            )
                        return o
                    tr = gsum(0, 4, "rc_tr")
                    r = ew.tile([128, 4, 16], f16, tag="rc_r")
                    nc.scalar.activation(r[:], tr[:], AF.Sigmoid)
                    tz = gsum(4, 8, "rc_tz")
                    z = ew.tile([128, 4, 16], f16, tag="rc_z")
                    zb = ew.tile([128, 4, 16], f16, tag="rc_zb")
                    nc.scalar.activation(z[:], tz[:], AF.Sigmoid)
                    nc.scalar.activation(zb[:], tz[:], AF.Sigmoid, scale=-1.0)
                    u = ew.tile([128, 4, 16], f32, tag="rc_u")
                    for q in range(4):
                        nc.vector.scalar_tensor_tensor(
                            out=u[:, q, :], in0=pst[:, 8 + q, :],
                            scalar=nb_sb[:, q : q + 1], in1=r[:, q, :],
                            op0=OP.add, op1=OP.mult,
                        )
                    tn = ew.tile([128, 4, 16], f32, tag="rc_tn")
                    nc.vector.scalar_tensor_tensor(
                        out=tn[:], in0=u[:], scalar=1.0,
                        in1=xpt[:, 8:12, vs], op0=OP.mult, op1=OP.add,
                    )
                    if xpb_dram is not None:
                        nc.vector.scalar_tensor_tensor(
                            out=tn[:], in0=tn[:], scalar=1.0,
                            in1=xbt[:, 8:12, rvs], op0=OP.mult, op1=OP.add,
                        )
                    n = ew.tile([128, 4, 16], f16, tag="rc_n")
                    nc.scalar.activation(n[:], tn[:], AF.Tanh)
                    a = ew.tile([128, 4, 16], f16, tag="rc_a")
                    nc.vector.scalar_tensor_tensor(
                        out=a[:], in0=z[:], scalar=1.0, in1=h[:],
                        op0=OP.mult, op1=OP.mult,
                    )
                    b2 = ew.tile([128, 4, 16], f16, tag="rc_b2")
                    nc.vector.scalar_tensor_tensor(
                        out=b2[:], in0=zb[:], scalar=1.0, in1=n[:],
                        op0=OP.mult, op1=OP.mult,
                    )
                    hn = hp.tile([128, 4, 16], f16, tag="rc_h")
                    nc.vector.scalar_tensor_tensor(
                        out=hn[:], in0=a[:], scalar=1.0, in1=b2[:],
                        op0=OP.mult, op1=OP.add,
                    )
                    if pooled is not None:
                        nc.vector.scalar_tensor_tensor(
                            out=pooled[:], in0=pooled[:], scalar=1.0, in1=hn[:],
                            op0=OP.mult, op1=OP.max,
                        )
                    if y_dram is not None:
                        nc.vector.tensor_copy(out=yb[:, :, vs], in_=hn[:])
                        if v == SBLK - 1:
                            sl = slice(blk * 256, (blk + 1) * 256)
                            nc.sync.dma_start(out=y_dram[:, :, sl], in_=yb[:])
                    h = hn

            zb12 = wp.tile([128, 12], f32, tag="zbias")
            nc.vector.memset(zb12[:], 0.0)
            # ---- phase A: L0 phase-1 input projection ----
            xp_gemm(eT1, wih1_s, 3, bias1_s, xp1d, "xg1")
            # ---- L0 recurrence 1, with phase-B GEMM blocks interleaved ----
            sideB = {
                b * 32: [
                    (lambda bb: lambda: xp_gemm_blk(
                        bb, eT2, wih2_s, 3, bias2_s, xp2d))(b)
                ]
                for b in range(NBLK)
            }
            recurrence(whh1_s, xp1d, nb1_s, y_dram=y1d, side=sideB, tag="r1")
            # ---- L0 recurrence 2, with L1 projection blocks interleaved ----
            sideE = {}
            for k in range(1, 8):
                sideE[32 * k] = [
                    (lambda bb: lambda: xp_gemm_blk(
                        bb, y1d, wa_s, 4, biasL_s, xpad))(k - 1),
                    (lambda bb: lambda: xp_gemm_blk(
                        bb, y2d, wb_s, 4, zb12, xpbd))(k - 1),
                ]
            recurrence(whh2_s, xp2d, nb2_s, y_dram=y2d, side=sideE, tag="r2")
            xp_gemm_blk(7, y1d, wa_s, 4, biasL_s, xpad)
            xp_gemm_blk(7, y2d, wb_s, 4, zb12, xpbd)
            # ---- L1 recurrence with on-the-fly max pool ----
            pooled = wp.tile([128, 4, 16], f16, tag="pooled")
            nc.vector.memset(pooled[:], -60000.0)
            recurrence(whhL_s, xpad, nbL_s, xpb_dram=xpbd, pooled=pooled, tag="rL")
            # ---- head partial: W1_half @ pooled ----
            hd = ps.tile([128, 16], f32, tag="gemm_ps")
            for k in range(4):
                nc.tensor.matmul(
                    hd[:], w1h_s[:, k, :], pooled[:, k, :],
                    start=(k == 0), stop=(k == 3),
                )
            ho = io.tile([128, 16], f32, tag="head_sb")
            nc.vector.tensor_copy(out=ho[:], in_=hd[:])
            nc.sync.dma_start(out=headout[:], in_=ho[:])

    _split_multiwaits(nc, mybir)
    try:
        ents = getattr(tc, "_perfetto_entries", None)
        span = None
        if ents:
            # (tile_name, allocated_time, freed_time, space, bytes, addr, tag)
            starts = [e[1] for e in ents if e[1] is not None]
            ends = [e[2] if e[2] is not None else e[1] for e in ents]
            if starts and ends:
                span = int(max(ends) - min(starts))
        _CACHE["model_ns"] = span
    except Exception:
        _CACHE["model_ns"] = None
    return nc


def _prep_core_inputs(inputs, g, role):
    """Host-side sharding/layout prep for core (batch group g, role)."""
    f16 = np.float16
    x = np.asarray(inputs["x"]).astype(np.int64)
    emb = np.asarray(inputs["emb"], dtype=np.float32)
    embp = np.zeros((V, EP), dtype=np.float32)
    embp[:, :E] = emb

    xg = x[g * BL : (g + 1) * BL]                     # [16, 256]
    e = embp[xg]                                      # [16, 256, 384]
    # eT[:, t*16+b] = e[b, t]  -> [384, 4096]
    eT_f = np.ascontiguousarray(e.transpose(2, 1, 0).reshape(EP, NTOK))
    er = e[:, ::-1, :]
    eT_r = np.ascontiguousarray(er.transpose(2, 1, 0).reshape(EP, NTOK))

    def ktile(wT, kt):   # [K, G'] -> [128, kt, G']
        Kd, Gd = wT.shape
        assert Kd == kt * 128
        return np.ascontiguousarray(
            wT.reshape(kt, 128, Gd).transpose(1, 0, 2)
        ).astype(f16)

    def e3(eT):          # [384, NTOK] -> [128, 3, NTOK]
        return np.ascontiguousarray(
            eT.reshape(3, 128, NTOK).transpose(1, 0, 2)
        ).astype(f16)

    def biascols(b_ih, b_hh):
        bv = b_ih.copy()
        bv[: 2 * H] += b_hh[: 2 * H]                  # r,z get both biases
        cols = np.ascontiguousarray(bv.reshape(12, 128).T).astype(np.float32)
        nb = np.ascontiguousarray(
            b_hh[2 * H :].reshape(4, 128).T
        ).astype(np.float32)
        return cols, nb

    w_ih0 = np.asarray(inputs["w_ih0"], dtype=np.float32)
    w_hh0 = np.asarray(inputs["w_hh0"], dtype=np.float32)
    b_ih0 = np.asarray(inputs["b_ih0"], dtype=np.float32)
    b_hh0 = np.asarray(inputs["b_hh0"], dtype=np.float32)
    w_ih1 = np.asarray(inputs["w_ih1"], dtype=np.float32)
    w_hh1 = np.asarray(inputs["w_hh1"], dtype=np.float32)
    b_ih1 = np.asarray(inputs["b_ih1"], dtype=np.float32)
    b_hh1 = np.asarray(inputs["b_hh1"], dtype=np.float32)
    w1 = np.asarray(inputs["w1"], dtype=np.float32)

    d1, d2 = (0, 1) if role == 0 else (1, 0)          # phase-1 dir, phase-2 dir
    dL = role                                          # L1 direction
    own_half = slice(0, H) if role == 0 else slice(H, 2 * H)
    oth_half = slice(H, 2 * H) if role == 0 else slice(0, H)

    def wihT(d):
        w = np.zeros((G, EP), dtype=np.float32)
        w[:, :E] = w_ih0[d]
        return ktile(w.T, 3)

    b1c, n1c = biascols(b_ih0[d1], b_hh0[d1])
    b2c, n2c = biascols(b_ih0[d2], b_hh0[d2])
    bLc, nLc = biascols(b_ih1[dL], b_hh1[dL])

    m = {
        "eT1": e3(eT_f if role == 0 else eT_r),
        "eT2": e3(eT_r if role == 0 else eT_f),
        "wih1": wihT(d1),
        "wih2": wihT(d2),
        "whh1": ktile(w_hh0[d1].T, 4),
        "whh2": ktile(w_hh0[d2].T, 4),
        "wa": ktile(w_ih1[dL][:, own_half].T, 4),
        "wb": ktile(w_ih1[dL][:, oth_half].T, 4),
        "whhL": ktile(w_hh1[dL].T, 4),
        "bias1": b1c, "bias2": b2c, "biasL": bLc,
        "nb1": n1c, "nb2": n2c, "nbL": nLc,
        "w1h": ktile(w1[:, own_half].T, 4),
    }
    return m


def kernel(**inputs) -> np.ndarray:
    from concourse.bass_utils import run_bass_kernel_spmd

    if "nc" not in _CACHE:
        _CACHE["nc"] = _build_nc()
    nc = _CACHE["nc"]

    core_ids = list(range(8))
    in_maps = []
    for c in core_ids:
        g, role = c % 4, c // 4
        in_maps.append(_prep_core_inputs(inputs, g, role))

    res = run_bass_kernel_spmd(nc, in_maps, core_ids)
    _CACHE["last_res"] = res

    b1 = np.asarray(inputs["b1"], dtype=np.float32)
    w2 = np.asarray(inputs["w2"], dtype=np.float32)
    b2 = np.asarray(inputs["b2"], dtype=np.float32)
    out = np.zeros((B, 2), dtype=np.float32)
    for g in range(4):
        p = (
            res.results[g]["headout"].astype(np.float32)
            + res.results[g + 4]["headout"].astype(np.float32)
        )                                              # [128 hid, 16 batch]
        hid = np.maximum(p + b1[:, None], 0.0)
        logits = w2 @ hid + b2[:, None]                # [2, 16]
        out[g * BL : (g + 1) * BL] = logits.T
    return out

